# revision 7
# baseline (speedup 1.0000x reference)
"""ECE loss (equal-width 15-bin) for [1048576, 128] logits on 8 TRN2 NeuronCores.

Strategy (data-parallel over rows, per the sharding hint):
  Device, per core (N/8 = 131072 rows):
    - gpsimd-issued casting DMA streams y_pred from DRAM (f32) into SBUF
      as bf16 in [128 partitions, 128 rows, 128 classes] chunks (software
      DGE is the only path that converts on the fly; big chunks keep the
      Q7 descriptor prep off the critical path)
    - DVE: 4-level pairwise bf16 max tree (128 -> 8) + grouped reduce_max
      tail -> per-row max m~ (f32 out; bf16->f32 is exact). TensorTensor
      gets the 2-byte 2x_1p DVE rate (0.5 cyc/elem) while TensorReduce
      always runs 1 cyc/elem, so the tree form halves the reduce cost.
    - ACT: one batched exp per 32-row slice, bf16 in -> bf16 out
    - DVE: 4-level pairwise bf16 add tree + grouped f32 reduce_sum tail
      -> denominator U
  Host:
    conf = exp(m~)/U  (max softmax up to the bf16 input rounding);
    acc = (bf16(y_pred[r, y_true[r]]) == m~): m~ is an exact bf16 element
    of the row, so equality in the bf16 domain reproduces argmax==label.
    Then the 15-bin histogram and ECE reduction as in the reference.

Working fully in bf16 perturbs conf by ~0.4% which moves the final ECE
by ~2e-3 relative (simulated on the real inputs; gate is 2e-2). Engine
budget per core: ACT ~121us, DVE ~165us, DMA in 67.1MB ~165-235us ->
memory-bound. The f32-exact predecessor ran DVE at ~255us (250us span);
the all-f32 ACT/DVE-balanced version before that was 259-289us.
"""

import numpy as np

import concourse.bacc as bacc
import concourse.tile as tile
from concourse import mybir
from concourse.bass_utils import run_bass_kernel_spmd

N_CORES = 8
N = 1048576
C = 128
N_SHARD = N // N_CORES  # 131072
P = 128                 # SBUF partitions
T = N_SHARD // P        # 1024 rows handled per partition
N_BINS = 15
K_TREE = 4              # bf16 tree levels: 128 -> 8

# input DMA chunks (big: fewer SWDGE descriptors) and compute slices
def _dma_schedule():
    gs = [16, 16, 32, 64] + [128] * 6 + [64, 32, 32]
    assert sum(gs) == T
    out = []
    t0 = 0
    for g in gs:
        out.append((t0, g))
        t0 += g
    return out

def _compute_schedule():
    # warm-up taper at the start (DMA prefetch gets ahead) and at the end
    # (short drain tail); every slice nests inside one DMA chunk
    gs = [8] * 8 + [16] * 4 + [32] * 26 + [16] * 2 + [8] * 4
    assert sum(gs) == T
    out = []
    t0 = 0
    for g in gs:
        out.append((t0, g))
        t0 += g
    return out

DMA_SCHED = _dma_schedule()
COMP_SCHED = _compute_schedule()

_CACHE: dict = {}


def _build_bass():
    nc = bacc.Bacc(None, target_bir_lowering=False)
    x = nc.dram_tensor("x", [N_SHARD, C], mybir.dt.float32, kind="ExternalInput")
    m_out = nc.dram_tensor("m_out", [N_SHARD], mybir.dt.float32, kind="ExternalOutput")
    u_out = nc.dram_tensor("u_out", [N_SHARD], mybir.dt.float32, kind="ExternalOutput")

    # row r = p*T + t lives at [p, t]; per-partition runs in DRAM stay contiguous
    xv = x[:, :].rearrange("(p t) c -> p t c", p=P)
    mv = m_out[:].rearrange("(p t) -> p t", p=P)
    uv = u_out[:].rearrange("(p t) -> p t", p=P)

    with tile.TileContext(nc) as tc:
        with (
            tc.tile_pool(name="xin", bufs=3) as xin_pool,
            tc.tile_pool(name="exps", bufs=3) as exp_pool,
            tc.tile_pool(name="tree", bufs=2) as tree_pool,
            tc.tile_pool(name="stats", bufs=1) as stats_pool,
            nc.allow_low_precision("bf16 pipeline; ECE impact ~2e-3 rel, gate 2e-2"),
        ):
            m_all = stats_pool.tile([P, T], mybir.dt.float32)
            u_all = stats_pool.tile([P, T], mybir.dt.float32)

            # casting input DMAs (f32 DRAM -> bf16 SBUF), big chunks
            chunks = []  # (handle, t0, g)
            dma_iter = iter(DMA_SCHED)

            def issue_dma():
                try:
                    t0, g = next(dma_iter)
                except StopIteration:
                    return
                xt = xin_pool.tile([P, g, C], mybir.dt.bfloat16, tag="xt", name=f"xt{t0}")
                nc.gpsimd.dma_start(out=xt[:], in_=xv[:, t0 : t0 + g, :])
                chunks.append((xt, t0, g))

            # prefetch the first three chunks before compute starts
            issue_dma()
            issue_dma()
            issue_dma()

            flushed = 0
            for si, (t0, g) in enumerate(COMP_SCHED):
                # find the landed chunk containing [t0, t0+g)
                xt, c0, cg = next(
                    (h, a, b) for (h, a, b) in chunks if a <= t0 and t0 + g <= a + b
                )
                o = t0 - c0
                xs = xt[:, o : o + g, :]

                # bf16 pairwise max tree 128 -> 8, then f32 reduce_max tail
                src = xs
                w = C
                for lvl in range(K_TREE):
                    w //= 2
                    dst = tree_pool.tile([P, g, w], mybir.dt.bfloat16, tag=f"mx{lvl}")
                    nc.vector.tensor_tensor(
                        out=dst[:],
                        in0=src[:, :, 0:w],
                        in1=src[:, :, w : 2 * w],
                        op=mybir.AluOpType.max,
                    )
                    src = dst
                nc.vector.reduce_max(
                    out=m_all[:, t0 : t0 + g],
                    in_=src[:],
                    axis=mybir.AxisListType.X,
                )

                et = exp_pool.tile([P, g, C], mybir.dt.bfloat16, tag="et")
                nc.scalar.activation(
                    out=et[:],
                    in_=xs,
                    func=mybir.ActivationFunctionType.Exp,
                )
                src = et
                w = C
                for lvl in range(K_TREE):
                    w //= 2
                    dst = tree_pool.tile([P, g, w], mybir.dt.bfloat16, tag=f"s{lvl}")
                    nc.vector.tensor_tensor(
                        out=dst[:],
                        in0=src[:, :, 0:w],
                        in1=src[:, :, w : 2 * w],
                        op=mybir.AluOpType.add,
                    )
                    src = dst
                nc.vector.reduce_sum(
                    out=u_all[:, t0 : t0 + g],
                    in_=src[:],
                    axis=mybir.AxisListType.X,
                )
                # finished consuming this input chunk -> issue the next DMA
                if t0 + g == c0 + cg:
                    issue_dma()
                if si % 8 == 7 or si == len(COMP_SCHED) - 1:
                    nc.sync.dma_start(
                        out=mv[:, flushed : t0 + g], in_=m_all[:, flushed : t0 + g]
                    )
                    nc.sync.dma_start(
                        out=uv[:, flushed : t0 + g], in_=u_all[:, flushed : t0 + g]
                    )
                    flushed = t0 + g
    nc.finalize()
    return nc


def run_device(y_pred: np.ndarray, **spmd_kwargs):
    """Run the bass kernel on 8 cores; returns (m~, U) each [N] f32 plus results obj."""
    if "nc" not in _CACHE:
        _CACHE["nc"] = _build_bass()
    nc = _CACHE["nc"]
    in_maps = [{"x": y_pred[c * N_SHARD : (c + 1) * N_SHARD]} for c in range(N_CORES)]
    res = run_bass_kernel_spmd(nc, in_maps, core_ids=list(range(N_CORES)), **spmd_kwargs)
    m = np.concatenate([r["m_out"] for r in res.results])
    u = np.concatenate([r["u_out"] for r in res.results])
    return m, u, res


def _bf16_rne(a: np.ndarray) -> np.ndarray:
    """Round f32 -> bf16 (round-to-nearest-even) and back to f32, in numpy."""
    u = a.astype(np.float32).view(np.uint32)
    rounded = (u + 0x7FFF + ((u >> 16) & 1)) & 0xFFFF0000
    return rounded.view(np.float32)


def finish_host(y_pred, y_true, m, u) -> np.ndarray:
    xl = y_pred[np.arange(N), np.asarray(y_true, dtype=np.int64)]
    conf = np.exp(m.astype(np.float64)) / u.astype(np.float64)
    # m is the max over the bf16-cast row: compare in the bf16 domain.
    # The DMA cast should round-to-nearest; fall back to truncation if the
    # returned maxima match that convention instead.
    xl_rne = _bf16_rne(xl)
    xl_trunc = (xl.astype(np.float32).view(np.uint32) & 0xFFFF0000).view(np.float32)
    acc_rne = xl_rne == m
    acc_trunc = xl_trunc == m
    # the true label is the argmax for the majority of rows in this data;
    # the matching convention is the one with the plausible (higher) hit rate
    acc = acc_rne if acc_rne.mean() >= acc_trunc.mean() else acc_trunc
    acc = acc.astype(np.float64)
    bin_idx = np.clip(np.ceil(conf * N_BINS).astype(np.int64) - 1, 0, N_BINS - 1)
    cnt = np.bincount(bin_idx, minlength=N_BINS).astype(np.float64)
    conf_sum = np.bincount(bin_idx, weights=conf, minlength=N_BINS)
    acc_sum = np.bincount(bin_idx, weights=acc, minlength=N_BINS)
    safe = np.where(cnt > 0, cnt, 1.0)
    per_bin = np.where(cnt > 0, np.abs(conf_sum / safe - acc_sum / safe) * (cnt / N), 0.0)
    return np.array([per_bin.sum()], dtype=np.float32)


def kernel(y_pred: np.ndarray, y_true: np.ndarray) -> np.ndarray:
    y_pred = np.ascontiguousarray(np.asarray(y_pred, dtype=np.float32))
    m, u, _ = run_device(y_pred)
    return finish_host(y_pred, y_true, m, u)


# revision 15
# speedup vs baseline: 1.1972x; 1.1972x over previous
"""ECE loss (equal-width 15-bin) for [1048576, 128] logits on 8 TRN2 NeuronCores.

Strategy (data-parallel over rows, per the sharding hint):
  Device, per core (N/8 = 131072 rows):
    - stream [128 partitions, G rows, 128 classes] supertiles of y_pred
    - DVE: grouped f32 reduce_max over classes -> per-row max m (exact)
    - ACT: one batched exp per supertile, written as bf16
    - DVE: 4-level pairwise bf16 add tree (128 -> 8) at the 2x_1p DVE
      rate (0.5 cyc/elem; TensorTensor gets the 2-byte perf mode, while
      TensorReduce always runs 1 cyc/elem), then one grouped f32
      reduce_sum over the last 8 -> denominator U
    - outputs m, U -- a 512MB -> 1MB reduction
  Host:
    conf = exp(m)/U  (== max softmax);  acc = (y_pred[r, y_true[r]] == m)
    (the row max is an exact element of the row, so float equality
    reproduces argmax == label up to exact-tie rows), then the 15-bin
    equal-width histogram and the final ECE reduction as in the reference.

The bf16 sum tree perturbs U by ~8e-4 rms which moves the final ECE by
~2e-5 relative (simulated on the real inputs; gate is 2e-2). Engine
budget per core: ACT ~110us, DVE ~137(max) + ~64(tree) + ~9(tail)
= ~210us, vs a DMA input stream of 67.1MB (~165us on fast cores,
~230us on the slowest). The previous ACT/DVE-balanced split ran both
engines at ~250us busy -> 259-289us/core.
"""

import numpy as np

import concourse.bacc as bacc
import concourse.tile as tile
from concourse import mybir
from concourse.bass_utils import run_bass_kernel_spmd

N_CORES = 8
N = 1048576
C = 128
N_SHARD = N // N_CORES  # 131072
P = 128                 # SBUF partitions
T = N_SHARD // P        # 1024 rows handled per partition
N_BINS = 15
K_TREE = 4              # bf16 tree levels: 128 -> 8

# warm-up schedule: small leading supertiles so compute starts ~8us earlier
# and the DMA prefetch queue stays ahead of compute from the start.
def _schedule():
    gs = [8] * 8 + [16] * 4 + [32] * 28
    assert sum(gs) == T
    sched = []
    t0 = 0
    for g in gs:
        sched.append((t0, g))
        t0 += g
    return sched

SCHED = _schedule()

_CACHE: dict = {}


def _build_bass():
    nc = bacc.Bacc(None, target_bir_lowering=False)
    x = nc.dram_tensor("x", [N_SHARD, C], mybir.dt.float32, kind="ExternalInput")
    m_out = nc.dram_tensor("m_out", [N_SHARD], mybir.dt.float32, kind="ExternalOutput")
    u_out = nc.dram_tensor("u_out", [N_SHARD], mybir.dt.float32, kind="ExternalOutput")

    # row r = p*T + t lives at [p, t]; per-partition runs in DRAM stay contiguous
    xv = x[:, :].rearrange("(p t) c -> p t c", p=P)
    mv = m_out[:].rearrange("(p t) -> p t", p=P)
    uv = u_out[:].rearrange("(p t) -> p t", p=P)

    with tile.TileContext(nc) as tc:
        with (
            tc.tile_pool(name="xin", bufs=8) as xin_pool,
            tc.tile_pool(name="exps", bufs=3) as exp_pool,
            tc.tile_pool(name="tree", bufs=2) as tree_pool,
            tc.tile_pool(name="stats", bufs=1) as stats_pool,
            nc.allow_low_precision("bf16 pairwise sum tree; ECE impact ~2e-5 rel"),
        ):
            m_all = stats_pool.tile([P, T], mybir.dt.float32)
            u_all = stats_pool.tile([P, T], mybir.dt.float32)
            flushed = 0
            for si, (t0, g) in enumerate(SCHED):
                xt = xin_pool.tile([P, g, C], mybir.dt.float32, tag="xt")
                nc.sync.dma_start(out=xt[:], in_=xv[:, t0 : t0 + g, :])
                nc.vector.reduce_max(
                    out=m_all[:, t0 : t0 + g],
                    in_=xt[:],
                    axis=mybir.AxisListType.X,
                )
                et = exp_pool.tile([P, g, C], mybir.dt.bfloat16, tag="et")
                nc.scalar.activation(
                    out=et[:],
                    in_=xt[:],
                    func=mybir.ActivationFunctionType.Exp,
                )
                # bf16 pairwise tree 128 -> 8 at the 2-byte DVE rate
                src = et
                w = C
                for lvl in range(K_TREE):
                    w //= 2
                    dst = tree_pool.tile([P, g, w], mybir.dt.bfloat16, tag=f"s{lvl}")
                    nc.vector.tensor_tensor(
                        out=dst[:],
                        in0=src[:, :, 0:w],
                        in1=src[:, :, w : 2 * w],
                        op=mybir.AluOpType.add,
                    )
                    src = dst
                nc.vector.reduce_sum(
                    out=u_all[:, t0 : t0 + g],
                    in_=src[:],
                    axis=mybir.AxisListType.X,
                )
                if si % 8 == 7 or si == len(SCHED) - 1:
                    nc.sync.dma_start(
                        out=mv[:, flushed : t0 + g], in_=m_all[:, flushed : t0 + g]
                    )
                    nc.sync.dma_start(
                        out=uv[:, flushed : t0 + g], in_=u_all[:, flushed : t0 + g]
                    )
                    flushed = t0 + g
    nc.finalize()
    return nc


def run_device(y_pred: np.ndarray, **spmd_kwargs):
    """Run the bass kernel on 8 cores; returns (m, U) each [N] f32 plus results obj."""
    if "nc" not in _CACHE:
        _CACHE["nc"] = _build_bass()
    nc = _CACHE["nc"]
    in_maps = [{"x": y_pred[c * N_SHARD : (c + 1) * N_SHARD]} for c in range(N_CORES)]
    res = run_bass_kernel_spmd(nc, in_maps, core_ids=list(range(N_CORES)), **spmd_kwargs)
    m = np.concatenate([r["m_out"] for r in res.results])
    u = np.concatenate([r["u_out"] for r in res.results])
    return m, u, res


def finish_host(y_pred, y_true, m, u) -> np.ndarray:
    xl = y_pred[np.arange(N), np.asarray(y_true, dtype=np.int64)]
    conf = np.exp(m.astype(np.float64)) / u.astype(np.float64)
    acc = (xl == m).astype(np.float64)
    bin_idx = np.clip(np.ceil(conf * N_BINS).astype(np.int64) - 1, 0, N_BINS - 1)
    cnt = np.bincount(bin_idx, minlength=N_BINS).astype(np.float64)
    conf_sum = np.bincount(bin_idx, weights=conf, minlength=N_BINS)
    acc_sum = np.bincount(bin_idx, weights=acc, minlength=N_BINS)
    safe = np.where(cnt > 0, cnt, 1.0)
    per_bin = np.where(cnt > 0, np.abs(conf_sum / safe - acc_sum / safe) * (cnt / N), 0.0)
    return np.array([per_bin.sum()], dtype=np.float32)


def kernel(y_pred: np.ndarray, y_true: np.ndarray) -> np.ndarray:
    y_pred = np.ascontiguousarray(np.asarray(y_pred, dtype=np.float32))
    m, u, _ = run_device(y_pred)
    return finish_host(y_pred, y_true, m, u)


# revision 17
# speedup vs baseline: 1.2324x; 1.0294x over previous
"""ECE loss (equal-width 15-bin) for [1048576, 128] logits on 8 TRN2 NeuronCores.

Strategy (data-parallel over rows, per the sharding hint):
  Device, per core (N/8 = 131072 rows):
    - stream [128 partitions, G rows, 128 classes] supertiles of y_pred
    - DVE: grouped f32 reduce_max over classes -> per-row max m (exact)
    - ACT: one batched exp per supertile, written as bf16
    - DVE: 4-level pairwise bf16 add tree (128 -> 8) at the 2x_1p DVE
      rate (0.5 cyc/elem; TensorTensor gets the 2-byte perf mode, while
      TensorReduce always runs 1 cyc/elem), then one grouped f32
      reduce_sum over the last 8 -> denominator U
    - outputs m, U -- a 512MB -> 1MB reduction
  Host:
    conf = exp(m)/U  (== max softmax);  acc = (y_pred[r, y_true[r]] == m)
    (the row max is an exact element of the row, so float equality
    reproduces argmax == label up to exact-tie rows), then the 15-bin
    equal-width histogram and the final ECE reduction as in the reference.

The bf16 sum tree perturbs U by ~8e-4 rms which moves the final ECE by
~2e-5 relative (simulated on the real inputs; gate is 2e-2). Engine
budget per core: ACT ~110us, DVE ~137(max) + ~64(tree) + ~9(tail)
= ~210us, vs a DMA input stream of 67.1MB (~165us on fast cores,
~230us on the slowest). The previous ACT/DVE-balanced split ran both
engines at ~250us busy -> 259-289us/core.
"""

import numpy as np

import concourse.bacc as bacc
import concourse.tile as tile
from concourse import mybir
from concourse.bass_utils import run_bass_kernel_spmd

N_CORES = 8
N = 1048576
C = 128
N_SHARD = N // N_CORES  # 131072
P = 128                 # SBUF partitions
T = N_SHARD // P        # 1024 rows handled per partition
N_BINS = 15
K_TREE = 4              # bf16 tree levels: 128 -> 8
KA32 = 6                # rows per 32 whose exp+sum runs fused on ACT (accum_out)

# warm-up schedule: small leading supertiles so compute starts ~8us earlier
# and the DMA prefetch queue stays ahead of compute from the start; small
# trailing ones shorten the post-last-byte drain chain.
def _schedule():
    gs = [8] * 8 + [16] * 4 + [32] * 26 + [16] * 2 + [8] * 4
    assert sum(gs) == T
    sched = []
    t0 = 0
    for g in gs:
        sched.append((t0, g, g * KA32 // 32))
        t0 += g
    return sched

SCHED = _schedule()

_CACHE: dict = {}


def _build_bass():
    nc = bacc.Bacc(None, target_bir_lowering=False)
    x = nc.dram_tensor("x", [N_SHARD, C], mybir.dt.float32, kind="ExternalInput")
    m_out = nc.dram_tensor("m_out", [N_SHARD], mybir.dt.float32, kind="ExternalOutput")
    u_out = nc.dram_tensor("u_out", [N_SHARD], mybir.dt.float32, kind="ExternalOutput")

    # row r = p*T + t lives at [p, t]; per-partition runs in DRAM stay contiguous
    xv = x[:, :].rearrange("(p t) c -> p t c", p=P)
    mv = m_out[:].rearrange("(p t) -> p t", p=P)
    uv = u_out[:].rearrange("(p t) -> p t", p=P)

    with tile.TileContext(nc) as tc:
        with (
            tc.tile_pool(name="xin", bufs=8) as xin_pool,
            tc.tile_pool(name="exps", bufs=3) as exp_pool,
            tc.tile_pool(name="tree", bufs=2) as tree_pool,
            tc.tile_pool(name="stats", bufs=1) as stats_pool,
            nc.allow_low_precision("bf16 pairwise sum tree; ECE impact ~2e-5 rel"),
        ):
            m_all = stats_pool.tile([P, T], mybir.dt.float32)
            u_all = stats_pool.tile([P, T], mybir.dt.float32)
            flushed = 0
            for si, (t0, g, ka) in enumerate(SCHED):
                kb = g - ka
                xt = xin_pool.tile([P, g, C], mybir.dt.float32, tag="xt")
                nc.sync.dma_start(out=xt[:], in_=xv[:, t0 : t0 + g, :])
                nc.vector.reduce_max(
                    out=m_all[:, t0 : t0 + g],
                    in_=xt[:],
                    axis=mybir.AxisListType.X,
                )
                # ACT path: rows [0, ka) get exp+sum fused via the f32
                # accumulator, written straight into u_all
                esc = exp_pool.tile([P, 1, C], mybir.dt.float32, tag="esc")
                for j in range(ka):
                    nc.scalar.activation(
                        out=esc[:],
                        in_=xt[:, j : j + 1, :],
                        func=mybir.ActivationFunctionType.Exp,
                        accum_out=u_all[:, t0 + j : t0 + j + 1],
                    )
                # DVE path: batched exp then bf16 pairwise tree at the
                # 2-byte DVE rate, f32 reduce tail
                et = exp_pool.tile([P, kb, C], mybir.dt.bfloat16, tag="et")
                nc.scalar.activation(
                    out=et[:],
                    in_=xt[:, ka:g, :],
                    func=mybir.ActivationFunctionType.Exp,
                )
                src = et
                w = C
                for lvl in range(K_TREE):
                    w //= 2
                    dst = tree_pool.tile([P, kb, w], mybir.dt.bfloat16, tag=f"s{lvl}")
                    nc.vector.tensor_tensor(
                        out=dst[:],
                        in0=src[:, :, 0:w],
                        in1=src[:, :, w : 2 * w],
                        op=mybir.AluOpType.add,
                    )
                    src = dst
                nc.vector.reduce_sum(
                    out=u_all[:, t0 + ka : t0 + g],
                    in_=src[:],
                    axis=mybir.AxisListType.X,
                )
                if si % 8 == 7 or si == len(SCHED) - 1:
                    nc.sync.dma_start(
                        out=mv[:, flushed : t0 + g], in_=m_all[:, flushed : t0 + g]
                    )
                    nc.sync.dma_start(
                        out=uv[:, flushed : t0 + g], in_=u_all[:, flushed : t0 + g]
                    )
                    flushed = t0 + g
    nc.finalize()
    return nc


def run_device(y_pred: np.ndarray, **spmd_kwargs):
    """Run the bass kernel on 8 cores; returns (m, U) each [N] f32 plus results obj."""
    if "nc" not in _CACHE:
        _CACHE["nc"] = _build_bass()
    nc = _CACHE["nc"]
    in_maps = [{"x": y_pred[c * N_SHARD : (c + 1) * N_SHARD]} for c in range(N_CORES)]
    res = run_bass_kernel_spmd(nc, in_maps, core_ids=list(range(N_CORES)), **spmd_kwargs)
    m = np.concatenate([r["m_out"] for r in res.results])
    u = np.concatenate([r["u_out"] for r in res.results])
    return m, u, res


def finish_host(y_pred, y_true, m, u) -> np.ndarray:
    xl = y_pred[np.arange(N), np.asarray(y_true, dtype=np.int64)]
    conf = np.exp(m.astype(np.float64)) / u.astype(np.float64)
    acc = (xl == m).astype(np.float64)
    bin_idx = np.clip(np.ceil(conf * N_BINS).astype(np.int64) - 1, 0, N_BINS - 1)
    cnt = np.bincount(bin_idx, minlength=N_BINS).astype(np.float64)
    conf_sum = np.bincount(bin_idx, weights=conf, minlength=N_BINS)
    acc_sum = np.bincount(bin_idx, weights=acc, minlength=N_BINS)
    safe = np.where(cnt > 0, cnt, 1.0)
    per_bin = np.where(cnt > 0, np.abs(conf_sum / safe - acc_sum / safe) * (cnt / N), 0.0)
    return np.array([per_bin.sum()], dtype=np.float32)


def kernel(y_pred: np.ndarray, y_true: np.ndarray) -> np.ndarray:
    y_pred = np.ascontiguousarray(np.asarray(y_pred, dtype=np.float32))
    m, u, _ = run_device(y_pred)
    return finish_host(y_pred, y_true, m, u)
